# revision 1
# baseline (speedup 1.0000x reference)
"""Coordinate multi-strip attention (pooling) kernel for 8 TRN2 NeuronCores.

Full inputs in, full outputs out. Data-parallel over batch B=32 -> 4
samples per core; all parameters replicated.

Algebraic folding done on host (all linear, exact up to fp reassociation):
  strip = mean_w(x)                      (raw sum; /64 folded into K)
  u     = (strip + dw3(strip) + dw7(strip)) / 3   (7-tap per-channel conv)
  u_bn  = (u - mean)*gamma/sqrt(var+eps) + beta   (affine per channel)
  y     = conv1_w @ concat(u_bn_h, u_bn_w)        (1x1 conv, contraction over C)
=>  y[m,l] = sum_{c,d} K[m,c,d] * strip_raw[c,l+d] + yb[m]
with K[m,c,d] = conv1_w[m,c] * wcomb[c,d] * bn_scale[c] / 64 and the bias
terms folded into the BN1 affine. The TensorEngine computes this as 7
shifted matmuls per channel-half, accumulating in PSUM.

Samples are processed in groups [0], [1,2], [3]: b0 solo so the gating
pipeline (GPSIMD) starts as early as possible; the middle pair batches
matmuls; b3 solo keeps the tail short.  The last two tiles' multiplies
are split h-wise across DVE and GPSIMD to halve the tail.

Engine assignment for the big streaming passes (GPSIMD contends with
2-port DVE ops for the shared SBUF port):
  ScalarE: transposed copy of each x tile so both strip reductions read
           dense (strided DVE reduce costs 7.0us vs 4.4us dense)
  DVE    : dense strip reductions + late g-builds
  GPSIMD : final x*g multiplies + b0's g-builds
"""

import numpy as np

import concourse.bass as bass
import concourse.mybir as mybir
import concourse.tile as tile
from concourse import bacc
from concourse.bass_utils import run_bass_kernel_spmd
from concourse.tile_rust import add_dep_helper

EPS = 1e-5
F32 = mybir.dt.float32
N_CORES = 8
B_LOCAL = 4  # 32 / 8
C = 256
H = 64
W = 64

_GROUPS = [[0], [1, 2], [3]]

# Per (b, cb): (g_build_engine, multiply_engine); 'v' = DVE, 'g' = gpsimd,
# 's' = multiply split h-wise across both engines.
# All g-builds on DVE with the a_h operand in PSUM: that keeps every DVE
# op at one SBUF read port, so GPSIMD streams at full speed concurrently.
_GATE_PLAN = {
    (0, 0): ('g', 'g'), (0, 1): ('g', 'g'),
    (1, 0): ('v', 'g'), (1, 1): ('v', 'g'),
    (2, 0): ('v', 'g'), (2, 1): ('v', 'g'),
    (3, 0): ('v', 'g'), (3, 1): ('v', 'g'),
}

_CACHE = {}


def _build_program():
    from contextlib import ExitStack

    nc = bacc.Bacc(
        "TRN2",
        target_bir_lowering=False,
        debug=False,
        enable_asserts=True,
        num_devices=N_CORES,
    )

    x_d = nc.dram_tensor("x", [B_LOCAL, C, H, W], F32, kind="ExternalInput")
    kt_d = nc.dram_tensor("kt", [2, 2, 128, 56], F32, kind="ExternalInput")
    wgt_d = nc.dram_tensor("wgt", [2, 8, 256], F32, kind="ExternalInput")
    sb_d = nc.dram_tensor("sb", [8, 8], F32, kind="ExternalInput")
    out_d = nc.dram_tensor("out", [B_LOCAL, C, H, W], F32, kind="ExternalOutput")

    mult = mybir.AluOpType.mult
    Relu = mybir.ActivationFunctionType.Relu
    Identity = mybir.ActivationFunctionType.Identity
    Sigmoid = mybir.ActivationFunctionType.Sigmoid
    Copy = mybir.ActivationFunctionType.Copy

    with tile.TileContext(nc) as tc, ExitStack() as ctx:
        const = ctx.enter_context(tc.tile_pool(name="const", bufs=1))
        xpool = ctx.enter_context(tc.tile_pool(name="xp", bufs=8))
        xtpool = ctx.enter_context(tc.tile_pool(name="xt", bufs=2))
        gpool = ctx.enter_context(tc.tile_pool(name="gp", bufs=2))
        strips = ctx.enter_context(tc.tile_pool(name="strips", bufs=1))
        vpool = ctx.enter_context(tc.tile_pool(name="vp", bufs=2))
        apool = ctx.enter_context(tc.tile_pool(name="ap", bufs=8))
        psum_y = ctx.enter_context(tc.tile_pool(name="py", bufs=2, space="PSUM"))
        psum_q = ctx.enter_context(tc.tile_pool(name="pq", bufs=2, space="PSUM"))
        psum_g = ctx.enter_context(tc.tile_pool(name="pg", bufs=4, space="PSUM"))

        # Constants
        kt_t = {}
        for dd in range(2):
            for cb in range(2):
                t = const.tile([128, 56], F32, tag=f"kt{dd}{cb}")
                nc.sync.dma_start(out=t[:], in_=kt_d[dd, cb])
                kt_t[dd, cb] = t
        wgt_t = {}
        for dd in range(2):
            t = const.tile([8, 256], F32, tag=f"wgt{dd}")
            nc.sync.dma_start(out=t[:], in_=wgt_d[dd])
            wgt_t[dd] = t
        sb_t = const.tile([8, 8], F32, tag="sb")
        nc.sync.dma_start(out=sb_t[:], in_=sb_d[:])

        # Strip tensors: [128c, 4b, 70] with 3-wide zero pads on both ends
        strip_t = {}
        for dd in range(2):
            for cb in range(2):
                t = strips.tile([128, B_LOCAL, 70], F32, tag=f"st{dd}{cb}")
                nc.gpsimd.memset(t[:, :, 0:3], 0.0)
                nc.gpsimd.memset(t[:, :, 67:70], 0.0)
                strip_t[dd, cb] = t

        X = {}
        A = {}
        prev_group_first_reduce = None
        for gi, bs in enumerate(_GROUPS):
            b0g, nb = bs[0], len(bs)
            first_reduce = None
            for b in bs:
                for cb in range(2):
                    t = xpool.tile([128, H, W], F32, tag="X")
                    # load in h-halves: first reduction can start earlier
                    for hh in range(2):
                        nc.sync.dma_start(
                            out=t[:, hh * 32:(hh + 1) * 32],
                            in_=x_d[b, cb * 128:(cb + 1) * 128,
                                    hh * 32:(hh + 1) * 32],
                        )
                    X[b, cb] = t
                last_group = gi == len(_GROUPS) - 1
                for cb in range(2):
                    xt = None if last_group else xtpool.tile(
                        [128, W, H], F32, tag="XT")
                    for hh in range(2):
                        hs = slice(hh * 32, (hh + 1) * 32)
                        r = nc.vector.reduce_sum(
                            out=strip_t[0, cb][:, b, 3 + hh * 32:3 + (hh + 1) * 32],
                            in_=X[b, cb][:, hs],
                            axis=mybir.AxisListType.X,
                        )
                        if first_reduce is None:
                            first_reduce = r
                        if not last_group:
                            nc.scalar.activation(
                                out=xt[:, :, hs].rearrange("p w h -> p h w"),
                                in_=X[b, cb][:, hs],
                                func=Copy,
                            )
                    if last_group:
                        # strided h-reduce straight off x: slower on DVE
                        # (7.0us vs 4.4) but skips the ACT-transpose chain,
                        # which sits on the tail's critical path
                        nc.vector.reduce_sum(
                            out=strip_t[1, cb][:, b, 3:67],
                            in_=X[b, cb][:].rearrange("p h w -> p w h"),
                            axis=mybir.AxisListType.X,
                        )
                    else:
                        nc.vector.reduce_sum(
                            out=strip_t[1, cb][:, b, 3:67],
                            in_=xt[:],
                            axis=mybir.AxisListType.X,
                        )
            prev_group_first_reduce = first_reduce

            # y_pre for the group: 7 shifted matmuls x 2 channel halves,
            # batched over the group's samples, PSUM-accumulated.
            yp = {}
            for dd in range(2):
                p = psum_y.tile([8, nb, 64], F32, tag="yp")
                n_mm = 0
                for cb in range(2):
                    for di in range(7):
                        nc.tensor.matmul(
                            p[:],
                            lhsT=kt_t[dd, cb][:, di * 8:(di + 1) * 8],
                            rhs=strip_t[dd, cb][:, b0g:b0g + nb, di:di + 64],
                            start=(n_mm == 0),
                            stop=(n_mm == 13),
                        )
                        n_mm += 1
                yp[dd] = p

            # BN1 + hswish:  z = s1*yp + b1_dir;  v = z * min(relu(z+3), 6)
            # q lives in PSUM so the min/mul avoid the SBUF port entirely.
            q = psum_q.tile([8, nb, 2, 64], F32, tag="q")
            v = vpool.tile([8, nb, 2, 64], F32, tag="v")
            for dd in range(2):
                nc.scalar.activation(
                    out=q[:, :, dd], in_=yp[dd][:], func=Relu,
                    scale=sb_t[:, 0:1], bias=sb_t[:, 3 + dd:4 + dd],
                )
                nc.scalar.activation(
                    out=v[:, :, dd], in_=yp[dd][:], func=Identity,
                    scale=sb_t[:, 0:1], bias=sb_t[:, 1 + dd:2 + dd],
                )
            nc.vector.tensor_scalar_min(q[:], q[:], 6.0)
            nc.vector.tensor_mul(v[:], v[:], q[:])

            # Gates: a = sigmoid(Wg/6 @ v), batched over the group
            for dd in range(2):
                for cb in range(2):
                    ga = psum_g.tile([128, nb, 64], F32, tag="ga")
                    nc.tensor.matmul(
                        ga[:],
                        lhsT=wgt_t[dd][:, cb * 128:(cb + 1) * 128],
                        rhs=v[:, :, dd],
                        start=True,
                        stop=True,
                    )
                    dve_g = all(
                        _GATE_PLAN[b, c][0] == 'v' for b in bs for c in range(2))
                    if dd == 0 and dve_g:
                        # a_h stays in PSUM: the g-build then reads it via
                        # the PSUM port, leaving SBUF rd1 free for GPSIMD
                        # (only valid when this group's g-builds run on DVE;
                        # GPSIMD has no PSUM access)
                        nc.scalar.activation(out=ga[:], in_=ga[:], func=Sigmoid)
                        A[gi, dd, cb] = ga
                    else:
                        at = apool.tile([128, nb, 64], F32, tag="a")
                        nc.scalar.activation(out=at[:], in_=ga[:], func=Sigmoid)
                        A[gi, dd, cb] = at

            # g = a_h (x) a_w ; X *= g ; store
            for b in bs:
                ip = b - b0g
                for cb in range(2):
                    g_eng, m_eng = _GATE_PLAN[b, cb]
                    g = gpool.tile([128, H, W], F32, tag="g")
                    ah_ap = A[gi, 0, cb][:, ip]  # [128, 64]
                    aw_ap = A[gi, 1, cb][:, ip]
                    ah = ah_ap.broadcast_to([128, H, W])  # [c, h, w*]
                    aw = bass.AP(
                        aw_ap.tensor, aw_ap.offset,
                        [list(aw_ap.ap[0]), [0, H], list(aw_ap.ap[1])],
                    )  # [c, h*, w]
                    eng1 = nc.vector if g_eng == 'v' else nc.gpsimd
                    eng1.tensor_tensor(g[:], ah, aw, mult)
                    xap = X[b, cb][:]
                    if m_eng == 's':
                        # split the multiply h-wise across both engines
                        nc.vector.tensor_tensor(
                            xap[:, 0:24], xap[:, 0:24], g[:, 0:24], mult)
                        nc.gpsimd.tensor_tensor(
                            xap[:, 24:64], xap[:, 24:64], g[:, 24:64], mult)
                        nc.sync.dma_start(
                            out=out_d[b, cb * 128:(cb + 1) * 128, 0:24],
                            in_=xap[:, 0:24])
                        nc.sync.dma_start(
                            out=out_d[b, cb * 128:(cb + 1) * 128, 24:64],
                            in_=xap[:, 24:64])
                    else:
                        eng2 = nc.vector if m_eng == 'v' else nc.gpsimd
                        eng2.tensor_tensor(xap, xap, g[:], mult)
                        nc.sync.dma_start(
                            out=out_d[b, cb * 128:(cb + 1) * 128], in_=xap)

    nc.compile()
    return nc


def _fold_strip_params(w3, w7, gamma, beta, mean, var):
    scale = gamma / np.sqrt(var + EPS)  # [C]
    wc = np.zeros((C, 7), np.float64)
    wc[:, 3] += 1.0
    wc[:, 2:5] += w3.astype(np.float64)
    wc[:, :] += w7.astype(np.float64)
    wc /= 3.0
    Wt = wc * scale[:, None].astype(np.float64) / 64.0  # [C, 7]
    bias_c = beta - mean * scale  # [C]
    return Wt, bias_c


def _pack_params(inp):
    conv1 = inp["conv1_w"].astype(np.float64)  # [8, 256]
    kt = np.zeros((2, 2, 128, 56), np.float32)
    sb = np.zeros((8, 8), np.float32)
    s1 = inp["bn1_gamma"] / np.sqrt(inp["bn1_var"] + EPS)  # [8]

    for dd, pre in enumerate(("sph", "spw")):
        Wt, bias_c = _fold_strip_params(
            inp[f"{pre}_w3"], inp[f"{pre}_w7"], inp[f"{pre}_gamma"],
            inp[f"{pre}_beta"], inp[f"{pre}_mean"], inp[f"{pre}_var"],
        )
        K = conv1[:, :, None] * Wt[None, :, :]  # [8, 256, 7]
        for cb in range(2):
            blk = K[:, cb * 128:(cb + 1) * 128, :]  # [8, 128, 7]
            kt[dd, cb] = blk.transpose(1, 2, 0).reshape(128, 56).astype(np.float32)
        yb = conv1 @ bias_c  # [8]
        b1 = (yb - inp["bn1_mean"]) * s1 + inp["bn1_beta"]  # [8]
        sb[:, 1 + dd] = b1.astype(np.float32)
        sb[:, 3 + dd] = (b1 + 3.0).astype(np.float32)

    sb[:, 0] = s1.astype(np.float32)

    wgt = np.zeros((2, 8, 256), np.float32)
    wgt[0] = (inp["convh_w"].T / 6.0).astype(np.float32)  # [m, o]
    wgt[1] = (inp["convw_w"].T / 6.0).astype(np.float32)
    return kt, wgt, sb


def kernel(**inputs):
    if "nc" not in _CACHE:
        _CACHE["nc"] = _build_program()
    nc = _CACHE["nc"]

    x = np.ascontiguousarray(inputs["x"], dtype=np.float32)
    kt, wgt, sb = _pack_params(inputs)

    in_maps = []
    for i in range(N_CORES):
        in_maps.append({
            "x": x[i * B_LOCAL:(i + 1) * B_LOCAL],
            "kt": kt,
            "wgt": wgt,
            "sb": sb,
        })
    res = run_bass_kernel_spmd(nc, in_maps, list(range(N_CORES)))
    out = np.concatenate([res.results[i]["out"] for i in range(N_CORES)], axis=0)
    return out



# revision 6
# speedup vs baseline: 1.0634x; 1.0634x over previous
"""Coordinate multi-strip attention (pooling) kernel for 8 TRN2 NeuronCores.

Full inputs in, full outputs out. Data-parallel over batch B=32 -> 4
samples per core; all parameters replicated.

v2: end-to-end bf16. x is converted to bf16 on the host (halves DMA-in),
the kernel computes in bf16 (2x DVE tensor_tensor throughput in the
packed 16-bit mode, 1-pass bf16 PE matmuls), and the output is written
to HBM as bf16 (halves DMA-out) then upcast to f32 on the host.
Measured end-to-end rel_l2 vs the f64 reference is ~4e-3 (gate: 2e-2).

Both strip reductions are pairwise tensor_tensor ADD TREES on
contiguous, step-1 slices (tensor_reduce is capped at 1x even for bf16;
tensor_tensor gets the 2x packed mode), so no transposed copy of x and
no strided reduce is needed:
  w-chain: s1 = x[..,0:32]+x[..,32:64]; s2 = halves(s1); reduce(16)
  h-chain: t1 = x[:,0:32,:]+x[:,32:64,:]; t2..t6 halve down to 1
Samples are processed in pairs so every op covers [128, 2b, ...].

Algebraic folding (host, f64): strip mean /64, the 3-conv mean /3, BN
scale into K[m,c,d]; BN bias chain into per-m bias; /6 of hswish into
the gate weights.

Engine split: DVE runs the w-chains, h-tree tails, and most multiplies;
GPSIMD runs the big h-tree heads (t1) and some m1 multiplies; ScalarE
runs the small activations/sigmoids; TensorE the (tiny) matmuls.
"""

import numpy as np
import ml_dtypes

import concourse.bass as bass
import concourse.mybir as mybir
import concourse.tile as tile
from concourse import bacc
from concourse.bass_utils import run_bass_kernel_spmd

EPS = 1e-5
F32 = mybir.dt.float32
BF16 = mybir.dt.bfloat16
NPBF = ml_dtypes.bfloat16
N_CORES = 8
B_LOCAL = 4  # 32 / 8
C = 256
H = 64
W = 64

# m1 (x *= ah) engine per (pair, cb, i): 'v' = DVE, 'g' = gpsimd
_M1_PLAN = {
    (0, 0, 0): 'g', (0, 0, 1): 'g',
    (0, 1, 0): 'g', (0, 1, 1): 'v',
    (1, 0, 0): 'g', (1, 0, 1): 'g',
    (1, 1, 0): 'v', (1, 1, 1): 'v',
}

_CACHE = {}


def _build_program():
    from contextlib import ExitStack

    nc = bacc.Bacc(
        "TRN2",
        target_bir_lowering=False,
        debug=False,
        enable_asserts=True,
        num_devices=N_CORES,
    )

    x_d = nc.dram_tensor("x", [B_LOCAL, C, H, W], BF16, kind="ExternalInput")
    kt_d = nc.dram_tensor("kt", [2, 2, 128, 56], BF16, kind="ExternalInput")
    wgt_d = nc.dram_tensor("wgt", [2, 8, 256], BF16, kind="ExternalInput")
    sb_d = nc.dram_tensor("sb", [8, 8], F32, kind="ExternalInput")
    out_d = nc.dram_tensor("out", [B_LOCAL, C, H, W], BF16, kind="ExternalOutput")

    mult = mybir.AluOpType.mult
    add = mybir.AluOpType.add
    Relu = mybir.ActivationFunctionType.Relu
    Identity = mybir.ActivationFunctionType.Identity
    Sigmoid = mybir.ActivationFunctionType.Sigmoid

    with tile.TileContext(nc) as tc, ExitStack() as ctx:
        ctx.enter_context(nc.allow_low_precision(
            reason="bf16 strip sums validated: end-to-end rel_l2 ~4e-3 "
                   "vs f64 reference (gate 2e-2)"))
        const = ctx.enter_context(tc.tile_pool(name="const", bufs=1))
        xpool = ctx.enter_context(tc.tile_pool(name="xp", bufs=4))
        tpool = ctx.enter_context(tc.tile_pool(name="tp", bufs=2))
        strips = ctx.enter_context(tc.tile_pool(name="strips", bufs=1))
        vpool = ctx.enter_context(tc.tile_pool(name="vp", bufs=2))
        apool = ctx.enter_context(tc.tile_pool(name="ap", bufs=8))
        psum_y = ctx.enter_context(tc.tile_pool(name="py", bufs=2, space="PSUM"))
        psum_q = ctx.enter_context(tc.tile_pool(name="pq", bufs=2, space="PSUM"))
        psum_g = ctx.enter_context(tc.tile_pool(name="pg", bufs=4, space="PSUM"))

        # ---- input DMA: first tile, then (tiny) consts, then the rest ----
        X = {}
        def load_x(pair, cb):
            t = xpool.tile([128, 2, H, W], BF16, tag="X")
            nc.sync.dma_start(
                out=t[:],
                in_=x_d[2 * pair:2 * pair + 2,
                        cb * 128:(cb + 1) * 128].rearrange(
                            "b c h w -> c b h w"),
            )
            X[pair, cb] = t

        load_x(0, 0)

        kt_t = {}
        for dd in range(2):
            for cb in range(2):
                t = const.tile([128, 56], BF16, tag=f"kt{dd}{cb}")
                nc.sync.dma_start(out=t[:], in_=kt_d[dd, cb])
                kt_t[dd, cb] = t
        wgt_t = {}
        for dd in range(2):
            t = const.tile([8, 256], BF16, tag=f"wgt{dd}")
            nc.sync.dma_start(out=t[:], in_=wgt_d[dd])
            wgt_t[dd] = t
        sb_t = const.tile([8, 8], F32, tag="sb")
        nc.sync.dma_start(out=sb_t[:], in_=sb_d[:])

        load_x(0, 1)
        load_x(1, 0)
        load_x(1, 1)

        # strip tensors [128, 4b, 70] bf16, 3-wide zero pads both ends
        strip_t = {}
        for dd in range(2):
            for cb in range(2):
                t = strips.tile([128, B_LOCAL, 70], BF16, tag=f"st{dd}{cb}")
                nc.gpsimd.memset(t[:, :, 0:3], 0.0)
                nc.gpsimd.memset(t[:, :, 67:70], 0.0)
                strip_t[dd, cb] = t

        A = {}
        for pair in range(2):
            bs = slice(2 * pair, 2 * pair + 2)
            # ---- strip reductions (tree adds, all step-1 contiguous) ----
            for cb in range(2):
                x_ = X[pair, cb][:]
                # w-chain (strip_h = sum over w): DVE
                s1 = tpool.tile([128, 2, H, 32], BF16, tag="s1")
                nc.vector.tensor_tensor(
                    s1[:], x_[:, :, :, 0:32], x_[:, :, :, 32:64], add)
                s2 = tpool.tile([128, 2, H, 16], BF16, tag="s2")
                nc.vector.tensor_tensor(
                    s2[:], s1[:, :, :, 0:16], s1[:, :, :, 16:32], add)
                nc.vector.reduce_sum(
                    out=strip_t[0, cb][:, bs, 3:67],
                    in_=s2[:],
                    axis=mybir.AxisListType.X,
                )
                # h-chain (strip_w = sum over h): t1 on GPSIMD, tail on DVE
                t1 = tpool.tile([128, 2, 32, W], BF16, tag="t1")
                nc.gpsimd.tensor_tensor(
                    t1[:], x_[:, :, 0:32], x_[:, :, 32:64], add)
                t2 = tpool.tile([128, 2, 16, W], BF16, tag="t2")
                nc.vector.tensor_tensor(
                    t2[:], t1[:, :, 0:16], t1[:, :, 16:32], add)
                t3 = tpool.tile([128, 2, 8, W], BF16, tag="t3")
                nc.vector.tensor_tensor(
                    t3[:], t2[:, :, 0:8], t2[:, :, 8:16], add)
                t4 = tpool.tile([128, 2, 4, W], BF16, tag="t4")
                nc.vector.tensor_tensor(
                    t4[:], t3[:, :, 0:4], t3[:, :, 4:8], add)
                t5 = tpool.tile([128, 2, 2, W], BF16, tag="t5")
                nc.vector.tensor_tensor(
                    t5[:], t4[:, :, 0:2], t4[:, :, 2:4], add)
                nc.vector.tensor_tensor(
                    strip_t[1, cb][:, bs, 3:67],
                    t5[:, :, 0], t5[:, :, 1], add)

            # ---- y_pre: 7 shifted matmuls x 2 channel halves, PSUM-acc ----
            yp = {}
            for dd in range(2):
                p = psum_y.tile([8, 2, 64], F32, tag="yp")
                n_mm = 0
                for cb in range(2):
                    for di in range(7):
                        nc.tensor.matmul(
                            p[:],
                            lhsT=kt_t[dd, cb][:, di * 8:(di + 1) * 8],
                            rhs=strip_t[dd, cb][:, bs, di:di + 64],
                            start=(n_mm == 0),
                            stop=(n_mm == 13),
                        )
                        n_mm += 1
                yp[dd] = p

            # ---- BN1 + hswish: z = s1*yp + b1; v = z*min(relu(z+3),6) ----
            q = psum_q.tile([8, 2, 2, 64], F32, tag="q")
            v = vpool.tile([8, 2, 2, 64], BF16, tag="v")
            for dd in range(2):
                nc.scalar.activation(
                    out=q[:, :, dd], in_=yp[dd][:], func=Relu,
                    scale=sb_t[:, 0:1], bias=sb_t[:, 3 + dd:4 + dd],
                )
            nc.vector.tensor_scalar_min(q[:], q[:], 6.0)
            for dd in range(2):
                nc.scalar.activation(
                    out=v[:, :, dd], in_=yp[dd][:], func=Identity,
                    scale=sb_t[:, 0:1], bias=sb_t[:, 1 + dd:2 + dd],
                )
            nc.vector.tensor_mul(v[:], v[:], q[:])

            # ---- gates: a = sigmoid(Wg/6 @ v) ----
            for dd in range(2):
                for cb in range(2):
                    ga = psum_g.tile([128, 2, 64], F32, tag="ga")
                    nc.tensor.matmul(
                        ga[:],
                        lhsT=wgt_t[dd][:, cb * 128:(cb + 1) * 128],
                        rhs=v[:, :, dd],
                        start=True,
                        stop=True,
                    )
                    at = apool.tile([128, 2, 64], BF16, tag="a")
                    nc.scalar.activation(out=at[:], in_=ga[:], func=Sigmoid)
                    A[pair, dd, cb] = at

            # ---- x *= ah; x *= aw; store ----
            for cb in range(2):
                xap = X[pair, cb][:]
                ah_t = A[pair, 0, cb]
                aw_t = A[pair, 1, cb]
                for i in range(2):
                    ah = ah_t[:, i].broadcast_to([128, H, W])  # [c,h,w*]
                    m1_eng = (nc.vector if _M1_PLAN[pair, cb, i] == 'v'
                              else nc.gpsimd)
                    m1_eng.tensor_tensor(xap[:, i], xap[:, i], ah, mult)
                for i in range(2):
                    awp = aw_t[:, i]
                    aw = bass.AP(
                        awp.tensor, awp.offset,
                        [list(awp.ap[0]), [0, H], list(awp.ap[1])],
                    )  # [c, h*, w]
                    nc.vector.tensor_tensor(xap[:, i], xap[:, i], aw, mult)
                    nc.sync.dma_start(
                        out=out_d[2 * pair + i, cb * 128:(cb + 1) * 128],
                        in_=xap[:, i])

    nc.compile()
    return nc


def _fold_strip_params(w3, w7, gamma, beta, mean, var):
    scale = gamma / np.sqrt(var + EPS)  # [C]
    wc = np.zeros((C, 7), np.float64)
    wc[:, 3] += 1.0
    wc[:, 2:5] += w3.astype(np.float64)
    wc[:, :] += w7.astype(np.float64)
    wc /= 3.0
    Wt = wc * scale[:, None].astype(np.float64) / 64.0  # [C, 7]
    bias_c = beta - mean * scale  # [C]
    return Wt, bias_c


def _pack_params(inp):
    conv1 = inp["conv1_w"].astype(np.float64)  # [8, 256]
    kt = np.zeros((2, 2, 128, 56), NPBF)
    sb = np.zeros((8, 8), np.float32)
    s1 = inp["bn1_gamma"] / np.sqrt(inp["bn1_var"] + EPS)  # [8]

    for dd, pre in enumerate(("sph", "spw")):
        Wt, bias_c = _fold_strip_params(
            inp[f"{pre}_w3"], inp[f"{pre}_w7"], inp[f"{pre}_gamma"],
            inp[f"{pre}_beta"], inp[f"{pre}_mean"], inp[f"{pre}_var"],
        )
        K = conv1[:, :, None] * Wt[None, :, :]  # [8, 256, 7]
        for cb in range(2):
            blk = K[:, cb * 128:(cb + 1) * 128, :]  # [8, 128, 7]
            kt[dd, cb] = blk.transpose(1, 2, 0).reshape(128, 56).astype(NPBF)
        yb = conv1 @ bias_c.astype(np.float64)  # [8]
        b1 = (yb - inp["bn1_mean"]) * s1 + inp["bn1_beta"]  # [8]
        sb[:, 1 + dd] = b1.astype(np.float32)
        sb[:, 3 + dd] = (b1 + 3.0).astype(np.float32)

    sb[:, 0] = s1.astype(np.float32)

    wgt = np.zeros((2, 8, 256), NPBF)
    wgt[0] = (inp["convh_w"].T / 6.0).astype(NPBF)  # [m, o]
    wgt[1] = (inp["convw_w"].T / 6.0).astype(NPBF)
    return kt, wgt, sb


def kernel(**inputs):
    if "nc" not in _CACHE:
        _CACHE["nc"] = _build_program()
    nc = _CACHE["nc"]

    x = np.ascontiguousarray(inputs["x"]).astype(NPBF)
    kt, wgt, sb = _pack_params(inputs)

    in_maps = []
    for i in range(N_CORES):
        in_maps.append({
            "x": x[i * B_LOCAL:(i + 1) * B_LOCAL],
            "kt": kt,
            "wgt": wgt,
            "sb": sb,
        })
    res = run_bass_kernel_spmd(nc, in_maps, list(range(N_CORES)))
    out = np.concatenate([res.results[i]["out"] for i in range(N_CORES)], axis=0)
    return out.astype(np.float32)


# revision 7
# speedup vs baseline: 1.3753x; 1.2933x over previous
"""Coordinate multi-strip attention (pooling) kernel for 8 TRN2 NeuronCores.

Full inputs in, full outputs out. Data-parallel over batch B=32 -> 4
samples per core; all parameters replicated.

v3: end-to-end bf16, single-engine (DVE) streaming. x is converted to
bf16 on the host (halves DMA-in), the kernel computes in bf16, and the
output is written to HBM as bf16 (halves DMA-out) then upcast on the
host. Measured end-to-end rel_l2 vs the f64 reference ~4e-3 (gate 2e-2).

Key empirical facts driving the design (from v1/v2 traces):
  - DVE tensor_tensor on bf16 step-1 4B-aligned operands hits the 2x
    packed mode: (N/2+151)/0.96 ns. tensor_reduce is capped at 1x.
  - A broadcast operand with inner step 0 (ah over w) drops TT to 1x;
    broadcast with inner step 1 (aw over h) keeps 2x.
  - DVE and GPSIMD running concurrent tensor_tensors degrade BOTH ~2x
    (shared SBUF ports) - GPSIMD is net-negative, so it only memsets.
Both strip reductions are pairwise tensor_tensor ADD TREES on
contiguous step-1 slices. The h-direction gate is expanded to full
[c,2b,h,w] by ScalarE directly out of PSUM (sigmoid with a stride-0
input AP), which keeps both multiplies fully step-1 on DVE.

Algebraic folding (host, f64): strip mean /64, the 3-conv mean /3, BN
scale into K[m,c,d]; BN bias chain into per-m bias; /6 of hswish into
the gate weights.
"""

import numpy as np
import ml_dtypes

import concourse.bass as bass
import concourse.mybir as mybir
import concourse.tile as tile
from concourse import bacc
from concourse.bass_utils import run_bass_kernel_spmd

EPS = 1e-5
F32 = mybir.dt.float32
BF16 = mybir.dt.bfloat16
NPBF = ml_dtypes.bfloat16
N_CORES = 8
B_LOCAL = 4  # 32 / 8
C = 256
H = 64
W = 64

_CACHE = {}


def _build_program():
    from contextlib import ExitStack

    nc = bacc.Bacc(
        "TRN2",
        target_bir_lowering=False,
        debug=False,
        enable_asserts=True,
        num_devices=N_CORES,
    )

    x_d = nc.dram_tensor("x", [B_LOCAL, C, H, W], BF16, kind="ExternalInput")
    kt_d = nc.dram_tensor("kt", [2, 2, 128, 56], BF16, kind="ExternalInput")
    wgt_d = nc.dram_tensor("wgt", [2, 8, 256], BF16, kind="ExternalInput")
    sb_d = nc.dram_tensor("sb", [8, 8], F32, kind="ExternalInput")
    out_d = nc.dram_tensor("out", [B_LOCAL, C, H, W], BF16, kind="ExternalOutput")

    mult = mybir.AluOpType.mult
    add = mybir.AluOpType.add
    Relu = mybir.ActivationFunctionType.Relu
    Identity = mybir.ActivationFunctionType.Identity
    Sigmoid = mybir.ActivationFunctionType.Sigmoid

    with tile.TileContext(nc) as tc, ExitStack() as ctx:
        ctx.enter_context(nc.allow_low_precision(
            reason="bf16 strip sums validated: end-to-end rel_l2 ~4e-3 "
                   "vs f64 reference (gate 2e-2)"))
        const = ctx.enter_context(tc.tile_pool(name="const", bufs=1))
        xpool = ctx.enter_context(tc.tile_pool(name="xp", bufs=4))
        tpool = ctx.enter_context(tc.tile_pool(name="tp", bufs=2))
        strips = ctx.enter_context(tc.tile_pool(name="strips", bufs=1))
        vpool = ctx.enter_context(tc.tile_pool(name="vp", bufs=2))
        apool = ctx.enter_context(tc.tile_pool(name="ap", bufs=4))
        gxpool = ctx.enter_context(tc.tile_pool(name="gx", bufs=2))
        psum_y = ctx.enter_context(tc.tile_pool(name="py", bufs=2, space="PSUM"))
        psum_q = ctx.enter_context(tc.tile_pool(name="pq", bufs=2, space="PSUM"))
        psum_g = ctx.enter_context(tc.tile_pool(name="pg", bufs=4, space="PSUM"))

        # ---- input DMA: first tile, then (tiny) consts, then the rest ----
        X = {}
        def load_x(pair, cb):
            t = xpool.tile([128, 2, H, W], BF16, tag="X")
            nc.sync.dma_start(
                out=t[:],
                in_=x_d[2 * pair:2 * pair + 2,
                        cb * 128:(cb + 1) * 128].rearrange(
                            "b c h w -> c b h w"),
            )
            X[pair, cb] = t

        load_x(0, 0)

        kt_t = {}
        for dd in range(2):
            for cb in range(2):
                t = const.tile([128, 56], BF16, tag=f"kt{dd}{cb}")
                nc.sync.dma_start(out=t[:], in_=kt_d[dd, cb])
                kt_t[dd, cb] = t
        wgt_t = {}
        for dd in range(2):
            t = const.tile([8, 256], BF16, tag=f"wgt{dd}")
            nc.sync.dma_start(out=t[:], in_=wgt_d[dd])
            wgt_t[dd] = t
        sb_t = const.tile([8, 8], F32, tag="sb")
        nc.sync.dma_start(out=sb_t[:], in_=sb_d[:])

        load_x(0, 1)
        load_x(1, 0)
        load_x(1, 1)

        # strip tensors [128, 4b, 70] bf16, 3-wide zero pads both ends
        strip_t = {}
        for dd in range(2):
            for cb in range(2):
                t = strips.tile([128, B_LOCAL, 70], BF16, tag=f"st{dd}{cb}")
                nc.gpsimd.memset(t[:, :, 0:3], 0.0)
                nc.gpsimd.memset(t[:, :, 67:70], 0.0)
                strip_t[dd, cb] = t

        for pair in range(2):
            bs = slice(2 * pair, 2 * pair + 2)
            # ---- strip reductions (tree adds, all step-1 contiguous) ----
            for cb in range(2):
                x_ = X[pair, cb][:]
                # w-chain (strip_h = sum over w)
                s1 = tpool.tile([128, 2, H, 32], BF16, tag="s1")
                nc.vector.tensor_tensor(
                    s1[:], x_[:, :, :, 0:32], x_[:, :, :, 32:64], add)
                s2 = tpool.tile([128, 2, H, 16], BF16, tag="s2")
                nc.vector.tensor_tensor(
                    s2[:], s1[:, :, :, 0:16], s1[:, :, :, 16:32], add)
                s3 = tpool.tile([128, 2, H, 8], BF16, tag="s3")
                nc.vector.tensor_tensor(
                    s3[:], s2[:, :, :, 0:8], s2[:, :, :, 8:16], add)
                nc.vector.reduce_sum(
                    out=strip_t[0, cb][:, bs, 3:67],
                    in_=s3[:],
                    axis=mybir.AxisListType.X,
                )
                # h-chain (strip_w = sum over h)
                t1 = tpool.tile([128, 2, 32, W], BF16, tag="t1")
                nc.vector.tensor_tensor(
                    t1[:], x_[:, :, 0:32], x_[:, :, 32:64], add)
                t2 = tpool.tile([128, 2, 16, W], BF16, tag="t2")
                nc.vector.tensor_tensor(
                    t2[:], t1[:, :, 0:16], t1[:, :, 16:32], add)
                t3 = tpool.tile([128, 2, 8, W], BF16, tag="t3")
                nc.vector.tensor_tensor(
                    t3[:], t2[:, :, 0:8], t2[:, :, 8:16], add)
                t4 = tpool.tile([128, 2, 4, W], BF16, tag="t4")
                nc.vector.tensor_tensor(
                    t4[:], t3[:, :, 0:4], t3[:, :, 4:8], add)
                t5 = tpool.tile([128, 2, 2, W], BF16, tag="t5")
                nc.vector.tensor_tensor(
                    t5[:], t4[:, :, 0:2], t4[:, :, 2:4], add)
                nc.vector.tensor_tensor(
                    strip_t[1, cb][:, bs, 3:67],
                    t5[:, :, 0], t5[:, :, 1], add)

            # ---- y_pre: 7 shifted matmuls x 2 channel halves, PSUM-acc ----
            yp = {}
            for dd in range(2):
                p = psum_y.tile([8, 2, 64], F32, tag="yp")
                n_mm = 0
                for cb in range(2):
                    for di in range(7):
                        nc.tensor.matmul(
                            p[:],
                            lhsT=kt_t[dd, cb][:, di * 8:(di + 1) * 8],
                            rhs=strip_t[dd, cb][:, bs, di:di + 64],
                            start=(n_mm == 0),
                            stop=(n_mm == 13),
                        )
                        n_mm += 1
                yp[dd] = p

            # ---- BN1 + hswish: z = s1*yp + b1; v = z*min(relu(z+3),6) ----
            q = psum_q.tile([8, 2, 2, 64], F32, tag="q")
            v = vpool.tile([8, 2, 2, 64], BF16, tag="v")
            for dd in range(2):
                nc.scalar.activation(
                    out=q[:, :, dd], in_=yp[dd][:], func=Relu,
                    scale=sb_t[:, 0:1], bias=sb_t[:, 3 + dd:4 + dd],
                )
            nc.vector.tensor_scalar_min(q[:], q[:], 6.0)
            for dd in range(2):
                nc.scalar.activation(
                    out=v[:, :, dd], in_=yp[dd][:], func=Identity,
                    scale=sb_t[:, 0:1], bias=sb_t[:, 1 + dd:2 + dd],
                )
            nc.vector.tensor_mul(v[:], v[:], q[:])

            # ---- gates ----
            # ah: sigmoid-EXPAND from PSUM to full [c, 2b, h, w] (ScalarE)
            # aw: compact sigmoid [c, 2b, w]; the m2 broadcast AP over h
            #     keeps inner step 1 so DVE stays in the 2x packed mode.
            AX = {}
            AW = {}
            for cb in range(2):
                ga = psum_g.tile([128, 2, 64], F32, tag="ga")
                nc.tensor.matmul(
                    ga[:], lhsT=wgt_t[0][:, cb * 128:(cb + 1) * 128],
                    rhs=v[:, :, 0], start=True, stop=True,
                )
                ax = gxpool.tile([128, 2, H, W], BF16, tag="ax")
                ga_b = ga[:].broadcast_to([128, 2, H, W])  # [c,b,h,w*]
                nc.scalar.activation(out=ax[:], in_=ga_b, func=Sigmoid)
                AX[cb] = ax
            for cb in range(2):
                ga = psum_g.tile([128, 2, 64], F32, tag="ga")
                nc.tensor.matmul(
                    ga[:], lhsT=wgt_t[1][:, cb * 128:(cb + 1) * 128],
                    rhs=v[:, :, 1], start=True, stop=True,
                )
                at = apool.tile([128, 2, 64], BF16, tag="a")
                nc.scalar.activation(out=at[:], in_=ga[:], func=Sigmoid)
                AW[cb] = at

            # ---- x *= ah_expanded; x *= aw_bcast; store ----
            for cb in range(2):
                xap = X[pair, cb][:]
                nc.vector.tensor_tensor(xap, xap, AX[cb][:], mult)
                awp = AW[cb][:]  # [128, 2, 64]
                aw = bass.AP(
                    awp.tensor, awp.offset,
                    [list(awp.ap[0]), list(awp.ap[1]), [0, H], list(awp.ap[2])],
                )  # [c, b, h*, w]
                nc.vector.tensor_tensor(xap, xap, aw, mult)
                for i in range(2):
                    nc.sync.dma_start(
                        out=out_d[2 * pair + i, cb * 128:(cb + 1) * 128],
                        in_=xap[:, i])

    nc.compile()
    return nc


def _fold_strip_params(w3, w7, gamma, beta, mean, var):
    scale = gamma / np.sqrt(var + EPS)  # [C]
    wc = np.zeros((C, 7), np.float64)
    wc[:, 3] += 1.0
    wc[:, 2:5] += w3.astype(np.float64)
    wc[:, :] += w7.astype(np.float64)
    wc /= 3.0
    Wt = wc * scale[:, None].astype(np.float64) / 64.0  # [C, 7]
    bias_c = beta - mean * scale  # [C]
    return Wt, bias_c


def _pack_params(inp):
    conv1 = inp["conv1_w"].astype(np.float64)  # [8, 256]
    kt = np.zeros((2, 2, 128, 56), NPBF)
    sb = np.zeros((8, 8), np.float32)
    s1 = inp["bn1_gamma"] / np.sqrt(inp["bn1_var"] + EPS)  # [8]

    for dd, pre in enumerate(("sph", "spw")):
        Wt, bias_c = _fold_strip_params(
            inp[f"{pre}_w3"], inp[f"{pre}_w7"], inp[f"{pre}_gamma"],
            inp[f"{pre}_beta"], inp[f"{pre}_mean"], inp[f"{pre}_var"],
        )
        K = conv1[:, :, None] * Wt[None, :, :]  # [8, 256, 7]
        for cb in range(2):
            blk = K[:, cb * 128:(cb + 1) * 128, :]  # [8, 128, 7]
            kt[dd, cb] = blk.transpose(1, 2, 0).reshape(128, 56).astype(NPBF)
        yb = conv1 @ bias_c.astype(np.float64)  # [8]
        b1 = (yb - inp["bn1_mean"]) * s1 + inp["bn1_beta"]  # [8]
        sb[:, 1 + dd] = b1.astype(np.float32)
        sb[:, 3 + dd] = (b1 + 3.0).astype(np.float32)

    sb[:, 0] = s1.astype(np.float32)

    wgt = np.zeros((2, 8, 256), NPBF)
    wgt[0] = (inp["convh_w"].T / 6.0).astype(NPBF)  # [m, o]
    wgt[1] = (inp["convw_w"].T / 6.0).astype(NPBF)
    return kt, wgt, sb


def kernel(**inputs):
    if "nc" not in _CACHE:
        _CACHE["nc"] = _build_program()
    nc = _CACHE["nc"]

    x = np.ascontiguousarray(inputs["x"]).astype(NPBF)
    kt, wgt, sb = _pack_params(inputs)

    in_maps = []
    for i in range(N_CORES):
        in_maps.append({
            "x": x[i * B_LOCAL:(i + 1) * B_LOCAL],
            "kt": kt,
            "wgt": wgt,
            "sb": sb,
        })
    res = run_bass_kernel_spmd(nc, in_maps, list(range(N_CORES)))
    out = np.concatenate([res.results[i]["out"] for i in range(N_CORES)], axis=0)
    return out.astype(np.float32)


# revision 9
# speedup vs baseline: 1.5764x; 1.1462x over previous
"""Coordinate multi-strip attention (pooling) kernel for 8 TRN2 NeuronCores.

Full inputs in, full outputs out. Data-parallel over batch B=32 -> 4
samples per core; all parameters replicated.

v4: end-to-end bf16, DVE-only streaming, latency-shaped pipeline.
x is converted to bf16 on the host (halves DMA-in), the kernel computes
in bf16, and the output is written to HBM as bf16 (halves DMA-out) then
upcast on the host. End-to-end rel_l2 vs the f64 reference ~4e-3
(gate 2e-2).

Empirical facts driving the design (v1-v3 traces):
  - DVE tensor_tensor, bf16, step-1, 4B-aligned operands hits the 2x
    packed mode: (N/2+151)/0.96 ns. tensor_reduce is capped at 1x.
  - A broadcast with inner step 0 (gate over w) drops TT to 1x, so the
    h-gate is expanded to full [c,h,w] by ScalarE straight out of PSUM
    (sigmoid with a stride-0 input AP); the w-gate broadcast keeps
    inner step 1 and stays 2x without expansion.
  - Concurrent DVE+GPSIMD tensor_tensors degrade BOTH ~2x (shared SBUF
    ports): GPSIMD would deliver 2.2 DVE-us per op while costing DVE
    ~4.5 us -> net negative. GPSIMD only memsets.
Both strip reductions are pairwise tensor_tensor ADD TREES on
contiguous step-1 slices. Pipeline shape per sample pair:
  w-chains -> [y_h matmuls + BN + gate + per-sample sigmoid-EXPAND on
  Tensor/Scalar] overlapped with h-chains on DVE -> per-sample m1 into
  a fresh tile -> [y_w chain + w-gate] -> per-sample m2 back into the
  x tile -> per-sample DMA out.

Algebraic folding (host, f64): strip mean /64, 3-conv mean /3, BN scale
into K[m,c,d]; BN bias chain into per-m bias; /6 of hswish into the
gate weights.
"""

import numpy as np
import ml_dtypes

import concourse.bass as bass
import concourse.mybir as mybir
import concourse.tile as tile
from concourse import bacc
from concourse.bass_utils import run_bass_kernel_spmd

EPS = 1e-5
F32 = mybir.dt.float32
BF16 = mybir.dt.bfloat16
NPBF = ml_dtypes.bfloat16
N_CORES = 8
B_LOCAL = 4  # 32 / 8
C = 256
H = 64
W = 64

_CACHE = {}


def _build_program():
    from contextlib import ExitStack

    nc = bacc.Bacc(
        "TRN2",
        target_bir_lowering=False,
        debug=False,
        enable_asserts=True,
        num_devices=N_CORES,
    )

    x_d = nc.dram_tensor("x", [B_LOCAL, C, H, W], BF16, kind="ExternalInput")
    kt_d = nc.dram_tensor("kt", [2, 2, 128, 56], BF16, kind="ExternalInput")
    wgt_d = nc.dram_tensor("wgt", [2, 8, 256], BF16, kind="ExternalInput")
    sb_d = nc.dram_tensor("sb", [8, 8], F32, kind="ExternalInput")
    out_d = nc.dram_tensor("out", [B_LOCAL, C, H, W], BF16, kind="ExternalOutput")

    mult = mybir.AluOpType.mult
    add = mybir.AluOpType.add
    Relu = mybir.ActivationFunctionType.Relu
    Identity = mybir.ActivationFunctionType.Identity
    Sigmoid = mybir.ActivationFunctionType.Sigmoid

    with tile.TileContext(nc) as tc, ExitStack() as ctx:
        ctx.enter_context(nc.allow_low_precision(
            reason="bf16 strip sums validated: end-to-end rel_l2 ~4e-3 "
                   "vs f64 reference (gate 2e-2)"))
        const = ctx.enter_context(tc.tile_pool(name="const", bufs=1))
        xpool = ctx.enter_context(tc.tile_pool(name="xp", bufs=4))
        tpool = ctx.enter_context(tc.tile_pool(name="tp", bufs=2))
        strips = ctx.enter_context(tc.tile_pool(name="strips", bufs=1))
        vpool = ctx.enter_context(tc.tile_pool(name="vp", bufs=2))
        apool = ctx.enter_context(tc.tile_pool(name="ap", bufs=4))
        gxpool = ctx.enter_context(tc.tile_pool(name="gx", bufs=4))
        mpool = ctx.enter_context(tc.tile_pool(name="mp", bufs=4))
        psum_y = ctx.enter_context(tc.tile_pool(name="py", bufs=2, space="PSUM"))
        psum_q = ctx.enter_context(tc.tile_pool(name="pq", bufs=2, space="PSUM"))
        psum_g = ctx.enter_context(tc.tile_pool(name="pg", bufs=4, space="PSUM"))

        # ---- input DMA: first tile per-sample, then consts, then rest ----
        X = {}
        x00 = xpool.tile([128, 2, H, W], BF16, tag="X")
        X[0, 0] = x00
        nc.sync.dma_start(out=x00[:, 0], in_=x_d[0, 0:128])

        kt_t = {}
        for dd in range(2):
            for cb in range(2):
                t = const.tile([128, 56], BF16, tag=f"kt{dd}{cb}")
                nc.sync.dma_start(out=t[:], in_=kt_d[dd, cb])
                kt_t[dd, cb] = t
        wgt_t = {}
        for dd in range(2):
            t = const.tile([8, 256], BF16, tag=f"wgt{dd}")
            nc.sync.dma_start(out=t[:], in_=wgt_d[dd])
            wgt_t[dd] = t
        sb_t = const.tile([8, 8], F32, tag="sb")
        nc.sync.dma_start(out=sb_t[:], in_=sb_d[:])

        nc.sync.dma_start(out=X[0, 0][:, 1], in_=x_d[1, 0:128])
        for pair, cb in ((0, 1), (1, 0), (1, 1)):
            t = xpool.tile([128, 2, H, W], BF16, tag="X")
            nc.sync.dma_start(
                out=t[:],
                in_=x_d[2 * pair:2 * pair + 2,
                        cb * 128:(cb + 1) * 128].rearrange(
                            "b c h w -> c b h w"),
            )
            X[pair, cb] = t

        # strip tensors [128, 4b, 70] bf16, 3-wide zero pads both ends
        strip_t = {}
        for dd in range(2):
            for cb in range(2):
                t = strips.tile([128, B_LOCAL, 70], BF16, tag=f"st{dd}{cb}")
                nc.gpsimd.memset(t[:, :, 0:3], 0.0)
                nc.gpsimd.memset(t[:, :, 67:70], 0.0)
                strip_t[dd, cb] = t

        def w_chain(pair, cb, i=None):
            # strip_h = sum over w via contiguous pairwise tree
            x_ = X[pair, cb][:]
            bsl = slice(2 * pair, 2 * pair + 2)
            nb = 2
            if i is not None:
                x_ = x_[:, i:i + 1]
                bsl = slice(2 * pair + i, 2 * pair + i + 1)
                nb = 1
            s1 = tpool.tile([128, nb, H, 32], BF16, tag=f"s1{nb}")
            nc.vector.tensor_tensor(
                s1[:], x_[:, :, :, 0:32], x_[:, :, :, 32:64], add)
            s2 = tpool.tile([128, nb, H, 16], BF16, tag=f"s2{nb}")
            nc.vector.tensor_tensor(
                s2[:], s1[:, :, :, 0:16], s1[:, :, :, 16:32], add)
            s3 = tpool.tile([128, nb, H, 8], BF16, tag=f"s3{nb}")
            nc.vector.tensor_tensor(
                s3[:], s2[:, :, :, 0:8], s2[:, :, :, 8:16], add)
            nc.vector.reduce_sum(
                out=strip_t[0, cb][:, bsl, 3:67],
                in_=s3[:],
                axis=mybir.AxisListType.X,
            )

        def h_chain(pair, cb):
            # strip_w = sum over h via contiguous pairwise tree
            x_ = X[pair, cb][:]
            bsl = slice(2 * pair, 2 * pair + 2)
            t1 = tpool.tile([128, 2, 32, W], BF16, tag="t1")
            nc.vector.tensor_tensor(
                t1[:], x_[:, :, 0:32], x_[:, :, 32:64], add)
            t2 = tpool.tile([128, 2, 16, W], BF16, tag="t2")
            nc.vector.tensor_tensor(
                t2[:], t1[:, :, 0:16], t1[:, :, 16:32], add)
            t3 = tpool.tile([128, 2, 8, W], BF16, tag="t3")
            nc.vector.tensor_tensor(
                t3[:], t2[:, :, 0:8], t2[:, :, 8:16], add)
            t4 = tpool.tile([128, 2, 4, W], BF16, tag="t4")
            nc.vector.tensor_tensor(
                t4[:], t3[:, :, 0:4], t3[:, :, 4:8], add)
            t5 = tpool.tile([128, 2, 2, W], BF16, tag="t5")
            nc.vector.tensor_tensor(
                t5[:], t4[:, :, 0:2], t4[:, :, 2:4], add)
            nc.vector.tensor_tensor(
                strip_t[1, cb][:, bsl, 3:67],
                t5[:, :, 0], t5[:, :, 1], add)

        def y_matmuls(pair, dd, cb, p, start):
            bsl = slice(2 * pair, 2 * pair + 2)
            for di in range(7):
                nc.tensor.matmul(
                    p[:],
                    lhsT=kt_t[dd, cb][:, di * 8:(di + 1) * 8],
                    rhs=strip_t[dd, cb][:, bsl, di:di + 64],
                    start=(start and di == 0),
                    stop=(not start and di == 6),
                )

        def bn_hswish(dd, yp):
            # z = s*yp + b1; v = z*min(relu(z+3), 6)  (/6 folded in gates)
            q = psum_q.tile([8, 2, 64], F32, tag="q")
            v = vpool.tile([8, 2, 64], BF16, tag="v")
            nc.scalar.activation(
                out=q[:], in_=yp[:], func=Relu,
                scale=sb_t[:, 0:1], bias=sb_t[:, 3 + dd:4 + dd],
            )
            nc.scalar.activation(
                out=v[:], in_=yp[:], func=Identity,
                scale=sb_t[:, 0:1], bias=sb_t[:, 1 + dd:2 + dd],
            )
            nc.vector.tensor_scalar_min(q[:], q[:], 6.0)
            nc.vector.tensor_mul(v[:], v[:], q[:])
            return v

        for pair in range(2):
            # ---- w-chains (per-sample for the very first tile) ----
            yp0 = psum_y.tile([8, 2, 64], F32, tag="yp")
            if pair == 0:
                w_chain(0, 0, i=0)
                w_chain(0, 0, i=1)
            else:
                w_chain(pair, 0)
            y_matmuls(pair, 0, 0, yp0, start=True)
            w_chain(pair, 1)
            y_matmuls(pair, 0, 1, yp0, start=False)

            # ---- h-gate: BN+hswish, gate matmul, per-sample expand ----
            v0 = bn_hswish(0, yp0)
            AX = {}
            for cb in range(2):
                ga = psum_g.tile([128, 2, 64], F32, tag="ga")
                nc.tensor.matmul(
                    ga[:], lhsT=wgt_t[0][:, cb * 128:(cb + 1) * 128],
                    rhs=v0[:], start=True, stop=True,
                )
                for i in range(2):
                    ax = gxpool.tile([128, H, W], BF16, tag="ax")
                    ga_b = ga[:, i].broadcast_to([128, H, W])  # [c,h,w*]
                    nc.scalar.activation(out=ax[:], in_=ga_b, func=Sigmoid)
                    AX[cb, i] = ax

            # ---- h-chains on DVE (overlap the gate pipeline above) ----
            yp1 = psum_y.tile([8, 2, 64], F32, tag="yp")
            h_chain(pair, 0)
            y_matmuls(pair, 1, 0, yp1, start=True)
            h_chain(pair, 1)
            y_matmuls(pair, 1, 1, yp1, start=False)

            # ---- m1 per sample: fresh tile = x * ah ----
            M = {}
            for cb in range(2):
                for i in range(2):
                    m = mpool.tile([128, H, W], BF16, tag="m")
                    nc.vector.tensor_tensor(
                        m[:], X[pair, cb][:, i], AX[cb, i][:], mult)
                    M[cb, i] = m

            # ---- w-gate (compact) ----
            v1 = bn_hswish(1, yp1)
            AW = {}
            for cb in range(2):
                ga = psum_g.tile([128, 2, 64], F32, tag="ga")
                nc.tensor.matmul(
                    ga[:], lhsT=wgt_t[1][:, cb * 128:(cb + 1) * 128],
                    rhs=v1[:], start=True, stop=True,
                )
                at = apool.tile([128, 2, 64], BF16, tag="a")
                nc.scalar.activation(out=at[:], in_=ga[:], func=Sigmoid)
                AW[cb] = at

            # ---- m2 per sample back into the x tile; DMA out ----
            for cb in range(2):
                for i in range(2):
                    awp = AW[cb][:, i]  # [128, 64]
                    aw = bass.AP(
                        awp.tensor, awp.offset,
                        [list(awp.ap[0]), [0, H], list(awp.ap[1])],
                    )  # [c, h*, w]
                    dst = X[pair, cb][:, i]
                    nc.vector.tensor_tensor(dst, M[cb, i][:], aw, mult)
                    nc.sync.dma_start(
                        out=out_d[2 * pair + i, cb * 128:(cb + 1) * 128],
                        in_=dst)

    nc.compile()
    return nc


def _fold_strip_params(w3, w7, gamma, beta, mean, var):
    scale = gamma / np.sqrt(var + EPS)  # [C]
    wc = np.zeros((C, 7), np.float64)
    wc[:, 3] += 1.0
    wc[:, 2:5] += w3.astype(np.float64)
    wc[:, :] += w7.astype(np.float64)
    wc /= 3.0
    Wt = wc * scale[:, None].astype(np.float64) / 64.0  # [C, 7]
    bias_c = beta - mean * scale  # [C]
    return Wt, bias_c


def _pack_params(inp):
    conv1 = inp["conv1_w"].astype(np.float64)  # [8, 256]
    kt = np.zeros((2, 2, 128, 56), NPBF)
    sb = np.zeros((8, 8), np.float32)
    s1 = inp["bn1_gamma"] / np.sqrt(inp["bn1_var"] + EPS)  # [8]

    for dd, pre in enumerate(("sph", "spw")):
        Wt, bias_c = _fold_strip_params(
            inp[f"{pre}_w3"], inp[f"{pre}_w7"], inp[f"{pre}_gamma"],
            inp[f"{pre}_beta"], inp[f"{pre}_mean"], inp[f"{pre}_var"],
        )
        K = conv1[:, :, None] * Wt[None, :, :]  # [8, 256, 7]
        for cb in range(2):
            blk = K[:, cb * 128:(cb + 1) * 128, :]  # [8, 128, 7]
            kt[dd, cb] = blk.transpose(1, 2, 0).reshape(128, 56).astype(NPBF)
        yb = conv1 @ bias_c.astype(np.float64)  # [8]
        b1 = (yb - inp["bn1_mean"]) * s1 + inp["bn1_beta"]  # [8]
        sb[:, 1 + dd] = b1.astype(np.float32)
        sb[:, 3 + dd] = (b1 + 3.0).astype(np.float32)

    sb[:, 0] = s1.astype(np.float32)

    wgt = np.zeros((2, 8, 256), NPBF)
    wgt[0] = (inp["convh_w"].T / 6.0).astype(NPBF)  # [m, o]
    wgt[1] = (inp["convw_w"].T / 6.0).astype(NPBF)
    return kt, wgt, sb


def kernel(**inputs):
    if "nc" not in _CACHE:
        _CACHE["nc"] = _build_program()
    nc = _CACHE["nc"]

    x = np.ascontiguousarray(inputs["x"]).astype(NPBF)
    kt, wgt, sb = _pack_params(inputs)

    in_maps = []
    for i in range(N_CORES):
        in_maps.append({
            "x": x[i * B_LOCAL:(i + 1) * B_LOCAL],
            "kt": kt,
            "wgt": wgt,
            "sb": sb,
        })
    res = run_bass_kernel_spmd(nc, in_maps, list(range(N_CORES)))
    out = np.concatenate([res.results[i]["out"] for i in range(N_CORES)], axis=0)
    return out.astype(np.float32)


# revision 11
# speedup vs baseline: 1.5847x; 1.0052x over previous
"""Coordinate multi-strip attention (pooling) kernel for 8 TRN2 NeuronCores.

Full inputs in, full outputs out. Data-parallel over batch B=32 -> 4
samples per core; all parameters replicated.

v4: end-to-end bf16, DVE-only streaming, latency-shaped pipeline.
x is converted to bf16 on the host (halves DMA-in), the kernel computes
in bf16, and the output is written to HBM as bf16 (halves DMA-out) then
upcast on the host. End-to-end rel_l2 vs the f64 reference ~4e-3
(gate 2e-2).

Empirical facts driving the design (v1-v3 traces):
  - DVE tensor_tensor, bf16, step-1, 4B-aligned operands hits the 2x
    packed mode: (N/2+151)/0.96 ns. tensor_reduce is capped at 1x.
  - A broadcast with inner step 0 (gate over w) drops TT to 1x, so the
    h-gate is expanded to full [c,h,w] by ScalarE straight out of PSUM
    (sigmoid with a stride-0 input AP); the w-gate broadcast keeps
    inner step 1 and stays 2x without expansion.
  - Concurrent DVE+GPSIMD tensor_tensors degrade BOTH ~2x (shared SBUF
    ports): GPSIMD would deliver 2.2 DVE-us per op while costing DVE
    ~4.5 us -> net negative. GPSIMD only memsets.
Both strip reductions are pairwise tensor_tensor ADD TREES on
contiguous step-1 slices. Pipeline shape per sample pair:
  w-chains -> [y_h matmuls + BN + gate + per-sample sigmoid-EXPAND on
  Tensor/Scalar] overlapped with h-chains on DVE -> per-sample m1 into
  a fresh tile -> [y_w chain + w-gate] -> per-sample m2 back into the
  x tile -> per-sample DMA out.

Algebraic folding (host, f64): strip mean /64, 3-conv mean /3, BN scale
into K[m,c,d]; BN bias chain into per-m bias; /6 of hswish into the
gate weights.
"""

import numpy as np
import ml_dtypes

import concourse.bass as bass
import concourse.mybir as mybir
import concourse.tile as tile
from concourse import bacc
from concourse.bass_utils import run_bass_kernel_spmd

EPS = 1e-5
F32 = mybir.dt.float32
BF16 = mybir.dt.bfloat16
NPBF = ml_dtypes.bfloat16
N_CORES = 8
B_LOCAL = 4  # 32 / 8
C = 256
H = 64
W = 64

_CACHE = {}


def _build_program():
    from contextlib import ExitStack

    nc = bacc.Bacc(
        "TRN2",
        target_bir_lowering=False,
        debug=False,
        enable_asserts=True,
        num_devices=N_CORES,
    )

    x_d = nc.dram_tensor("x", [B_LOCAL, C, H, W], BF16, kind="ExternalInput")
    kt_d = nc.dram_tensor("kt", [2, 2, 128, 56], BF16, kind="ExternalInput")
    wgt_d = nc.dram_tensor("wgt", [2, 8, 256], BF16, kind="ExternalInput")
    sb_d = nc.dram_tensor("sb", [8, 8], F32, kind="ExternalInput")
    out_d = nc.dram_tensor("out", [B_LOCAL, C, H, W], BF16, kind="ExternalOutput")

    mult = mybir.AluOpType.mult
    add = mybir.AluOpType.add
    Relu = mybir.ActivationFunctionType.Relu
    Identity = mybir.ActivationFunctionType.Identity
    Sigmoid = mybir.ActivationFunctionType.Sigmoid

    with tile.TileContext(nc) as tc, ExitStack() as ctx:
        ctx.enter_context(nc.allow_low_precision(
            reason="bf16 strip sums validated: end-to-end rel_l2 ~4e-3 "
                   "vs f64 reference (gate 2e-2)"))
        const = ctx.enter_context(tc.tile_pool(name="const", bufs=1))
        xpool = ctx.enter_context(tc.tile_pool(name="xp", bufs=4))
        tpool = ctx.enter_context(tc.tile_pool(name="tp", bufs=2))
        strips = ctx.enter_context(tc.tile_pool(name="strips", bufs=1))
        vpool = ctx.enter_context(tc.tile_pool(name="vp", bufs=2))
        apool = ctx.enter_context(tc.tile_pool(name="ap", bufs=4))
        gxpool = ctx.enter_context(tc.tile_pool(name="gx", bufs=4))
        mpool = ctx.enter_context(tc.tile_pool(name="mp", bufs=4))
        psum_y = ctx.enter_context(tc.tile_pool(name="py", bufs=2, space="PSUM"))
        psum_q = ctx.enter_context(tc.tile_pool(name="pq", bufs=2, space="PSUM"))
        psum_g = ctx.enter_context(tc.tile_pool(name="pg", bufs=4, space="PSUM"))

        # ---- input DMA: urgent x tiles first, consts slotted later ----
        X = {}
        x00 = xpool.tile([128, 2, H, W], BF16, tag="X")
        X[0, 0] = x00
        nc.sync.dma_start(out=x00[:, 0], in_=x_d[0, 0:128])
        nc.sync.dma_start(out=x00[:, 1], in_=x_d[1, 0:128])
        x01 = xpool.tile([128, 2, H, W], BF16, tag="X")
        X[0, 1] = x01
        nc.sync.dma_start(
            out=x01[:],
            in_=x_d[0:2, 128:256].rearrange("b c h w -> c b h w"))

        kt_t = {}
        for dd in range(2):
            for cb in range(2):
                t = const.tile([128, 56], BF16, tag=f"kt{dd}{cb}")
                nc.sync.dma_start(out=t[:], in_=kt_d[dd, cb])
                kt_t[dd, cb] = t
        wgt_t = {}
        for dd in range(2):
            t = const.tile([8, 256], BF16, tag=f"wgt{dd}")
            nc.sync.dma_start(out=t[:], in_=wgt_d[dd])
            wgt_t[dd] = t
        sb_t = const.tile([8, 8], F32, tag="sb")
        nc.sync.dma_start(out=sb_t[:], in_=sb_d[:])

        for cb in range(2):
            t = xpool.tile([128, 2, H, W], BF16, tag="X")
            nc.sync.dma_start(
                out=t[:],
                in_=x_d[2:4, cb * 128:(cb + 1) * 128].rearrange(
                    "b c h w -> c b h w"),
            )
            X[1, cb] = t

        # strip tensors [128, 4b, 70] bf16, 3-wide zero pads both ends
        strip_t = {}
        for dd in range(2):
            for cb in range(2):
                t = strips.tile([128, B_LOCAL, 70], BF16, tag=f"st{dd}{cb}")
                nc.gpsimd.memset(t[:, :, 0:3], 0.0)
                nc.gpsimd.memset(t[:, :, 67:70], 0.0)
                strip_t[dd, cb] = t

        def w_chain(pair, cb, i=None):
            # strip_h = sum over w via contiguous pairwise tree
            x_ = X[pair, cb][:]
            bsl = slice(2 * pair, 2 * pair + 2)
            nb = 2
            if i is not None:
                x_ = x_[:, i:i + 1]
                bsl = slice(2 * pair + i, 2 * pair + i + 1)
                nb = 1
            s1 = tpool.tile([128, nb, H, 32], BF16, tag=f"s1{nb}")
            nc.vector.tensor_tensor(
                s1[:], x_[:, :, :, 0:32], x_[:, :, :, 32:64], add)
            s2 = tpool.tile([128, nb, H, 16], BF16, tag=f"s2{nb}")
            nc.vector.tensor_tensor(
                s2[:], s1[:, :, :, 0:16], s1[:, :, :, 16:32], add)
            s3 = tpool.tile([128, nb, H, 8], BF16, tag=f"s3{nb}")
            nc.vector.tensor_tensor(
                s3[:], s2[:, :, :, 0:8], s2[:, :, :, 8:16], add)
            nc.vector.reduce_sum(
                out=strip_t[0, cb][:, bsl, 3:67],
                in_=s3[:],
                axis=mybir.AxisListType.X,
            )

        def h_chain(pair, cb):
            # strip_w = sum over h via contiguous pairwise tree
            x_ = X[pair, cb][:]
            bsl = slice(2 * pair, 2 * pair + 2)
            t1 = tpool.tile([128, 2, 32, W], BF16, tag="t1")
            nc.vector.tensor_tensor(
                t1[:], x_[:, :, 0:32], x_[:, :, 32:64], add)
            t2 = tpool.tile([128, 2, 16, W], BF16, tag="t2")
            nc.vector.tensor_tensor(
                t2[:], t1[:, :, 0:16], t1[:, :, 16:32], add)
            t3 = tpool.tile([128, 2, 8, W], BF16, tag="t3")
            nc.vector.tensor_tensor(
                t3[:], t2[:, :, 0:8], t2[:, :, 8:16], add)
            t4 = tpool.tile([128, 2, 4, W], BF16, tag="t4")
            nc.vector.tensor_tensor(
                t4[:], t3[:, :, 0:4], t3[:, :, 4:8], add)
            t5 = tpool.tile([128, 2, 2, W], BF16, tag="t5")
            nc.vector.tensor_tensor(
                t5[:], t4[:, :, 0:2], t4[:, :, 2:4], add)
            nc.vector.tensor_tensor(
                strip_t[1, cb][:, bsl, 3:67],
                t5[:, :, 0], t5[:, :, 1], add)

        def y_matmuls(pair, dd, cb, p, start):
            bsl = slice(2 * pair, 2 * pair + 2)
            for di in range(7):
                nc.tensor.matmul(
                    p[:],
                    lhsT=kt_t[dd, cb][:, di * 8:(di + 1) * 8],
                    rhs=strip_t[dd, cb][:, bsl, di:di + 64],
                    start=(start and di == 0),
                    stop=(not start and di == 6),
                )

        def bn_hswish(dd, yp):
            # z = s*yp + b1; v = z*min(relu(z+3), 6)  (/6 folded in gates)
            q = psum_q.tile([8, 2, 64], F32, tag="q")
            v = vpool.tile([8, 2, 64], BF16, tag="v")
            nc.scalar.activation(
                out=q[:], in_=yp[:], func=Relu,
                scale=sb_t[:, 0:1], bias=sb_t[:, 3 + dd:4 + dd],
            )
            nc.scalar.activation(
                out=v[:], in_=yp[:], func=Identity,
                scale=sb_t[:, 0:1], bias=sb_t[:, 1 + dd:2 + dd],
            )
            nc.vector.tensor_scalar_min(q[:], q[:], 6.0)
            nc.vector.tensor_mul(v[:], v[:], q[:])
            return v

        def w_phase(pair):
            # w-chains + y_h matmul chains
            yp0 = psum_y.tile([8, 2, 64], F32, tag="yp")
            if pair == 0:
                w_chain(0, 0, i=0)
                w_chain(0, 0, i=1)
            else:
                w_chain(pair, 0)
            y_matmuls(pair, 0, 0, yp0, start=True)
            w_chain(pair, 1)
            y_matmuls(pair, 0, 1, yp0, start=False)
            return yp0

        def hgate_phase(pair, yp0):
            # BN+hswish, gate matmul, per-sample sigmoid-EXPAND (ScalarE)
            v0 = bn_hswish(0, yp0)
            AX = {}
            for cb in range(2):
                ga = psum_g.tile([128, 2, 64], F32, tag="ga")
                nc.tensor.matmul(
                    ga[:], lhsT=wgt_t[0][:, cb * 128:(cb + 1) * 128],
                    rhs=v0[:], start=True, stop=True,
                )
                for i in range(2):
                    ax = gxpool.tile([128, H, W], BF16, tag="ax")
                    ga_b = ga[:, i].broadcast_to([128, H, W])  # [c,h,w*]
                    nc.scalar.activation(out=ax[:], in_=ga_b, func=Sigmoid)
                    AX[cb, i] = ax
            return AX

        def h_phase(pair):
            # h-chains + y_w matmul chains
            yp1 = psum_y.tile([8, 2, 64], F32, tag="yp")
            h_chain(pair, 0)
            y_matmuls(pair, 1, 0, yp1, start=True)
            h_chain(pair, 1)
            y_matmuls(pair, 1, 1, yp1, start=False)
            return yp1

        def m1_phase(pair, AX):
            M = {}
            for cb in range(2):
                for i in range(2):
                    m = mpool.tile([128, H, W], BF16, tag="m")
                    nc.vector.tensor_tensor(
                        m[:], X[pair, cb][:, i], AX[cb, i][:], mult)
                    M[cb, i] = m
            return M

        def wgate_phase(pair, yp1):
            v1 = bn_hswish(1, yp1)
            AW = {}
            for cb in range(2):
                ga = psum_g.tile([128, 2, 64], F32, tag="ga")
                nc.tensor.matmul(
                    ga[:], lhsT=wgt_t[1][:, cb * 128:(cb + 1) * 128],
                    rhs=v1[:], start=True, stop=True,
                )
                at = apool.tile([128, 2, 64], BF16, tag="a")
                nc.scalar.activation(out=at[:], in_=ga[:], func=Sigmoid)
                AW[cb] = at
            return AW

        def m2_phase(pair, M, AW):
            for cb in range(2):
                for i in range(2):
                    awp = AW[cb][:, i]  # [128, 64]
                    aw = bass.AP(
                        awp.tensor, awp.offset,
                        [list(awp.ap[0]), [0, H], list(awp.ap[1])],
                    )  # [c, h*, w]
                    dst = X[pair, cb][:, i]
                    nc.vector.tensor_tensor(dst, M[cb, i][:], aw, mult)
                    nc.sync.dma_start(
                        out=out_d[2 * pair + i, cb * 128:(cb + 1) * 128],
                        in_=dst)

        # Interleaved 2-pair schedule: pair-1 tree work fills the DVE
        # while pair-0's gate pipeline (Tensor/Scalar) is in flight, and
        # vice versa, so the endgame is pure multiplies.
        yp0_a = w_phase(0)
        AX_a = hgate_phase(0, yp0_a)
        yp1_a = h_phase(0)
        yp0_b = w_phase(1)
        M_a = m1_phase(0, AX_a)
        AW_a = wgate_phase(0, yp1_a)
        AX_b = hgate_phase(1, yp0_b)
        yp1_b = h_phase(1)
        m2_phase(0, M_a, AW_a)
        M_b = m1_phase(1, AX_b)
        AW_b = wgate_phase(1, yp1_b)
        m2_phase(1, M_b, AW_b)

    nc.compile()
    return nc


def _fold_strip_params(w3, w7, gamma, beta, mean, var):
    scale = gamma / np.sqrt(var + EPS)  # [C]
    wc = np.zeros((C, 7), np.float64)
    wc[:, 3] += 1.0
    wc[:, 2:5] += w3.astype(np.float64)
    wc[:, :] += w7.astype(np.float64)
    wc /= 3.0
    Wt = wc * scale[:, None].astype(np.float64) / 64.0  # [C, 7]
    bias_c = beta - mean * scale  # [C]
    return Wt, bias_c


def _pack_params(inp):
    conv1 = inp["conv1_w"].astype(np.float64)  # [8, 256]
    kt = np.zeros((2, 2, 128, 56), NPBF)
    sb = np.zeros((8, 8), np.float32)
    s1 = inp["bn1_gamma"] / np.sqrt(inp["bn1_var"] + EPS)  # [8]

    for dd, pre in enumerate(("sph", "spw")):
        Wt, bias_c = _fold_strip_params(
            inp[f"{pre}_w3"], inp[f"{pre}_w7"], inp[f"{pre}_gamma"],
            inp[f"{pre}_beta"], inp[f"{pre}_mean"], inp[f"{pre}_var"],
        )
        K = conv1[:, :, None] * Wt[None, :, :]  # [8, 256, 7]
        for cb in range(2):
            blk = K[:, cb * 128:(cb + 1) * 128, :]  # [8, 128, 7]
            kt[dd, cb] = blk.transpose(1, 2, 0).reshape(128, 56).astype(NPBF)
        yb = conv1 @ bias_c.astype(np.float64)  # [8]
        b1 = (yb - inp["bn1_mean"]) * s1 + inp["bn1_beta"]  # [8]
        sb[:, 1 + dd] = b1.astype(np.float32)
        sb[:, 3 + dd] = (b1 + 3.0).astype(np.float32)

    sb[:, 0] = s1.astype(np.float32)

    wgt = np.zeros((2, 8, 256), NPBF)
    wgt[0] = (inp["convh_w"].T / 6.0).astype(NPBF)  # [m, o]
    wgt[1] = (inp["convw_w"].T / 6.0).astype(NPBF)
    return kt, wgt, sb


def kernel(**inputs):
    if "nc" not in _CACHE:
        _CACHE["nc"] = _build_program()
    nc = _CACHE["nc"]

    x = np.ascontiguousarray(inputs["x"]).astype(NPBF)
    kt, wgt, sb = _pack_params(inputs)

    in_maps = []
    for i in range(N_CORES):
        in_maps.append({
            "x": x[i * B_LOCAL:(i + 1) * B_LOCAL],
            "kt": kt,
            "wgt": wgt,
            "sb": sb,
        })
    res = run_bass_kernel_spmd(nc, in_maps, list(range(N_CORES)))
    out = np.concatenate([res.results[i]["out"] for i in range(N_CORES)], axis=0)
    return out.astype(np.float32)
